# revision 7
# baseline (speedup 1.0000x reference)
"""Distributed Trainium2 kernel for nn_Criterion_35012573397697.

Proxy-NCA-style loss: mean_b[ d(x_b, p_{y_b}) + logsumexp_{c != y_b}(-d(x_b, p_c)) ]
with x = 3*l2norm(batch), p = 3*l2norm(proxies), d = squared euclidean.

Strategy (8 NeuronCores, classes sharded; d(x,p) = 18 - 2*s with s = (3x^)(3p^)):
  - pshard is shipped host-side in tile-major layout [c%128, t, d] so each
    DMA chunk is per-partition contiguous (full-rate HBM streaming).
  - 7 chunks of 16/12 tiles, each chunk aligned 1:1 with an "exp group"
    (2048/1536 classes x 4 b-tiles) so the whole pipeline overlaps:
    gpsimd squares -> DVE reduce/rsqrt -> DVE bf16 copy + scale (2x modes)
    -> PE transpose (psum bank pool) -> DVE drain -> PE matmuls (512-col
    slabs into a dedicated double-buffered exp psum pool) -> ScalarE
    exp(2s-18) with fused accumulation.
  - One tiny AllGather of [128, 4] partial sums; every core computes the
    final scalar identically. Positive-class distance computed exactly in
    f32 from host-gathered proxies[labels] rows (indexing only).
"""

import math

import numpy as np
import ml_dtypes

import concourse.bass as bass
import concourse.bacc as bacc
import concourse.mybir as mybir
import concourse.tile as tile
from concourse import bass_isa
from concourse.bass_utils import run_bass_kernel_spmd

N_CORES = 8
B = 512
D = 128
C = 100000
SH = 12800           # padded shard size per core
NT = SH // 128       # 100 c-tiles of 128
BT = B // 128        # 4 b-tiles
PAD_ROWS = N_CORES * SH - C   # 2400 zero rows in total
PAD_CORR = PAD_ROWS * math.exp(-18.0)

# chunk sizes in tiles; exp groups are 12 tiles (1536 classes), last 4-tile tail
TCHUNKS = [12, 12, 12, 12, 12, 12, 12, 16]
GBOUND = [0, 12, 24, 36, 48, 60, 72, 84, 96, 100]
NG = len(GBOUND) - 1   # 9 groups per b-tile

F32 = mybir.dt.float32
BF16 = mybir.dt.bfloat16
AX = mybir.AxisListType
OP = mybir.AluOpType
AF = mybir.ActivationFunctionType

_CACHE = {}


def _rsqrt_dve(nc, pool, dst, src, n, scale=1.0):
    """dst = scale / sqrt(src) via Quake III bit trick + 2 Newton steps."""
    I32 = mybir.dt.int32
    v = pool.tile([128, n], F32, tag="rsq_v")
    nc.vector.tensor_scalar(v[:], src, 1e-12, None, OP.max)
    src = v[:]
    h = pool.tile([128, n], I32, tag="rsq_h")
    nc.vector.tensor_scalar(h[:], src.bitcast(I32), 1, None,
                            OP.logical_shift_right)
    y0 = pool.tile([128, n], I32, tag="rsq_y0")
    nc.vector.tensor_scalar(y0[:], h[:], -1, 0x5F3759DF, OP.mult, OP.add)
    y0f = y0[:].bitcast(F32)
    t = pool.tile([128, n], F32, tag="rsq_t")
    y1 = pool.tile([128, n], F32, tag="rsq_y1")
    nc.vector.tensor_tensor(t[:], y0f, y0f, OP.mult)        # y0^2
    nc.vector.tensor_tensor(t[:], t[:], src, OP.mult)       # v*y0^2
    nc.vector.tensor_scalar(t[:], t[:], -0.5, 1.5, OP.mult, OP.add)
    nc.vector.tensor_tensor(y1[:], y0f, t[:], OP.mult)      # Newton 1
    nc.vector.tensor_tensor(t[:], y1[:], y1[:], OP.mult)    # y1^2
    nc.vector.tensor_tensor(t[:], t[:], src, OP.mult)       # v*y1^2
    nc.vector.tensor_scalar(t[:], t[:], -0.5 * scale, 1.5 * scale,
                            OP.mult, OP.add)
    nc.vector.tensor_tensor(dst, y1[:], t[:], OP.mult)      # Newton 2


def build_graph():
    nc = bacc.Bacc("TRN2", target_bir_lowering=False, debug=False,
                   num_devices=N_CORES)
    # tile-major proxy shard: [c%128, t, d]
    p_ext = nc.dram_tensor("pshard", [128, NT, D], F32, kind="ExternalInput").ap()
    b_ext = nc.dram_tensor("batch", [B, D], F32, kind="ExternalInput").ap()
    sel_ext = nc.dram_tensor("psel", [B, D], F32, kind="ExternalInput").ap()
    id_ext = nc.dram_tensor("ident", [128, 128], BF16, kind="ExternalInput").ap()
    out_ext = nc.dram_tensor("out", [1, 1], F32, kind="ExternalOutput").ap()

    gcum = GBOUND

    with tile.TileContext(nc) as tc:
        with tc.tile_pool(name="dram", bufs=1, space="DRAM") as dram, \
             tc.tile_pool(name="big", bufs=1) as bigp, \
             tc.tile_pool(name="sb", bufs=2) as pool, \
             tc.tile_pool(name="psA", bufs=2, space="PSUM") as psA, \
             tc.tile_pool(name="psT", bufs=2, space="PSUM") as psT:

            # ---------- DMA loads: first proxy chunk first ----------
            praw = bigp.tile([128, NT, 128], F32)       # [c%128, t, d]
            nc.sync.dma_start(praw[:, 0:TCHUNKS[0], :], p_ext[:, 0:TCHUNKS[0], :])

            xb = bigp.tile([128, BT, 128], F32)         # [b%128, bt, d]
            nc.sync.dma_start(xb[:], b_ext.rearrange("(t p) d -> p t d", p=128))
            selb = bigp.tile([128, BT, 128], F32)
            nc.sync.dma_start(selb[:], sel_ext.rearrange("(t p) d -> p t d", p=128))
            ident = bigp.tile([128, 128], BF16)
            nc.sync.dma_start(ident[:], id_ext[:])

            clo = TCHUNKS[0]
            for ck in TCHUNKS[1:]:
                nc.sync.dma_start(praw[:, clo:clo + ck, :],
                                  p_ext[:, clo:clo + ck, :])
                clo += ck

            # early tiny AllGather absorbs first-collective warm-up.
            dag_in = dram.tile([1, 16], F32)
            dag_out = dram.tile([N_CORES, 16], F32)
            z16 = bigp.tile([1, 16], F32)
            nc.vector.memset(z16[:], 0.0)
            dagj = bigp.tile([1, 1], F32)
            nc.sync.dma_start(dag_in[:], z16[:])
            nc.gpsimd.collective_compute(
                "AllGather", OP.bypass,
                replica_groups=[list(range(N_CORES))],
                ins=[dag_in.opt()], outs=[dag_out.opt()],
            )
            nc.sync.dma_start(dagj[:], dag_out[0:1, 0:1])

            bias18 = bigp.tile([128, 1], F32)
            nc.vector.memset(bias18[:], -18.0)

            # ---------- x / psel prep (tiny, on DVE) ----------
            n2 = bigp.tile([128, 2 * BT], F32)
            sqx = pool.tile([128, BT, 128], F32, tag="sqscr")
            nc.vector.tensor_tensor(sqx[:], xb[:], xb[:], OP.mult)
            nc.vector.tensor_reduce(n2[:, 0:BT], sqx[:], axis=AX.X, op=OP.add)
            sqs = pool.tile([128, BT, 128], F32, tag="sqscr")
            nc.vector.tensor_tensor(sqs[:], selb[:], selb[:], OP.mult)
            nc.vector.tensor_reduce(n2[:, BT:2 * BT], sqs[:], axis=AX.X,
                                    op=OP.add)
            rn = bigp.tile([128, 2 * BT], F32)   # 1/sqrt(n2)
            _rsqrt_dve(nc, pool, rn[:], n2[:], 2 * BT)

            posdot = bigp.tile([128, BT], F32)
            sqd = pool.tile([128, BT, 128], F32, tag="sqscr")
            nc.vector.tensor_tensor(sqd[:], xb[:], selb[:], OP.mult)
            nc.vector.tensor_reduce(posdot[:], sqd[:], axis=AX.X, op=OP.add)
            posd = bigp.tile([128, BT], F32)
            tmp4 = pool.tile([128, BT], F32, tag="smallscr")
            nc.vector.tensor_tensor(tmp4[:], posdot[:], rn[:, 0:BT], OP.mult)
            nc.vector.tensor_tensor(tmp4[:], tmp4[:], rn[:, BT:2 * BT], OP.mult)
            nc.vector.tensor_scalar(posd[:], tmp4[:], -18.0, 18.0, OP.mult,
                                    OP.add)

            xscale3 = bigp.tile([128, BT], F32)
            nc.vector.tensor_scalar_mul(xscale3[:], rn[:, 0:BT], 3.0)
            xhat = bigp.tile([128, BT, 128], BF16)
            for t in range(BT):
                nc.vector.tensor_scalar_mul(xhat[:, t, :], xb[:, t, :],
                                            xscale3[:, t:t + 1])
            xT = bigp.tile([128, BT, 128], BF16)
            xps = psT.tile([128, 8, 128], BF16, tag="tp")
            for t in range(BT):
                nc.tensor.transpose(xps[:, t, :], xhat[:, t, :], ident[:])
            nc.vector.tensor_copy(xT[:], xps[:, 0:BT, :])

            # ---------- per-chunk proxy pipeline ----------
            pn2 = bigp.tile([128, NT], F32)
            pscale3 = bigp.tile([128, NT], F32)
            pbf = bigp.tile([128, NT, 128], BF16)     # 3*normalized, bf16
            pT = bigp.tile([128, NT, 128], BF16)      # [d, t, c%128]
            partials = bigp.tile([128, BT * NG], F32)
            pTf = pT[:].rearrange("p t c -> p (t c)")

            def issue_group(g, bt):
                lo_c, hi_c = gcum[g] * 128, gcum[g + 1] * 128
                width = hi_c - lo_c
                sp = psA.tile([128, 1536], F32, tag="exp")
                for j in range(width // 512):
                    nc.tensor.matmul(
                        sp[:, j * 512:(j + 1) * 512],
                        xT[:, bt, :],
                        pTf[:, lo_c + j * 512: lo_c + (j + 1) * 512],
                        start=True, stop=True)
                ej = pool.tile([128, 1536], BF16, tag="ejunk")
                nc.scalar.activation(
                    ej[:, 0:width], sp[:, 0:width], AF.Exp,
                    bias=bias18[:, 0:1], scale=2.0,
                    accum_out=partials[:, bt * NG + g:bt * NG + g + 1])

            clo = 0
            for gi, ck in enumerate(TCHUNKS):
                lo, hi = clo, clo + ck
                clo += ck
                # squares on gpsimd (frees DVE)
                psq = pool.tile([128, 16, 128], F32, tag="psq")
                nc.gpsimd.tensor_tensor(psq[:, 0:ck, :], praw[:, lo:hi, :],
                                        praw[:, lo:hi, :], OP.mult)
                nc.vector.tensor_reduce(pn2[:, lo:hi], psq[:, 0:ck, :],
                                        axis=AX.X, op=OP.add)
                _rsqrt_dve(nc, pool, pscale3[:, lo:hi], pn2[:, lo:hi], ck,
                           scale=3.0)
                # f32 -> bf16 copy at 2x, then bf16 scale at 2x
                pbraw = pool.tile([128, 16, 128], BF16, tag="pbraw")
                nc.vector.tensor_copy(pbraw[:, 0:ck, :], praw[:, lo:hi, :])
                nc.vector.tensor_tensor(
                    pbf[:, lo:hi, :], pbraw[:, 0:ck, :],
                    pscale3[:, lo:hi, None].to_broadcast((128, ck, 128)),
                    OP.mult)
                # transpose via PE in groups of 8 tiles, drain on DVE
                for g0 in range(lo, hi, 8):
                    w = min(8, hi - g0)
                    tp = psT.tile([128, 8, 128], BF16, tag="tp")
                    for j in range(w):
                        nc.tensor.transpose(tp[:, j, :], pbf[:, g0 + j, :],
                                            ident[:])
                    nc.vector.tensor_copy(pT[:, g0:g0 + w, :], tp[:, 0:w, :])
                # matmuls + fused exp/accum for ready groups
                glist = [gi] if gi < 7 else [7, 8]
                for g in glist:
                    for bt in range(BT):
                        issue_group(g, bt)

            # ---------- local partial sums, tiny AllGather ----------
            s_loc = bigp.tile([128, BT], F32)
            nc.vector.tensor_reduce(
                s_loc[:], partials[:].rearrange("p (t g) -> p t g", t=BT),
                axis=AX.X, op=OP.add)

            ag_in = dram.tile([128, BT], F32)
            ag_out = dram.tile([128 * N_CORES, BT], F32)
            nc.sync.dma_start(ag_in[:], s_loc[:])
            nc.gpsimd.collective_compute(
                "AllGather", OP.bypass,
                replica_groups=[list(range(N_CORES))],
                ins=[ag_in.opt()], outs=[ag_out.opt()],
            )
            gath = bigp.tile([128, BT, N_CORES], F32)
            nc.sync.dma_start(gath[:],
                              ag_out.rearrange("(r p) f -> p f r", p=128))
            s_tot = bigp.tile([128, BT], F32)
            nc.vector.tensor_reduce(s_tot[:], gath[:], axis=AX.X, op=OP.add)

            # ---------- finale (identical on every core) ----------
            npos = pool.tile([128, BT], F32, tag="fin")
            nc.scalar.activation(npos[:], posd[:], AF.Exp, scale=-1.0)
            s1 = pool.tile([128, BT], F32, tag="fin")
            nc.vector.tensor_scalar(s1[:], s_tot[:], -float(PAD_CORR),
                                    None, OP.add)
            nc.vector.tensor_tensor(s1[:], s1[:], npos[:], OP.subtract)
            lse = pool.tile([128, BT], F32, tag="fin")
            nc.scalar.activation(lse[:], s1[:], AF.Ln)
            perb = pool.tile([128, BT], F32, tag="fin")
            nc.vector.tensor_tensor(perb[:], posd[:], lse[:], OP.add)
            csum = pool.tile([128, 1], F32, tag="fin")
            nc.vector.tensor_reduce(csum[:], perb[:], axis=AX.X, op=OP.add)
            nc.vector.tensor_tensor(csum[0:1, 0:1], csum[0:1, 0:1],
                                    dagj[:], OP.add)
            allred = pool.tile([128, 1], F32, tag="fin")
            nc.gpsimd.partition_all_reduce(allred[:], csum[:], channels=128,
                                           reduce_op=bass_isa.ReduceOp.add)
            res = pool.tile([1, 1], F32, tag="fin")
            nc.vector.tensor_scalar_mul(res[:], allred[0:1, 0:1], 1.0 / B)
            nc.sync.dma_start(out_ext[:], res[:])

    nc.compile()
    return nc


def make_in_maps(batch, labels, proxies):
    batch = np.ascontiguousarray(batch, dtype=np.float32)
    labels = np.asarray(labels).astype(np.int64)
    proxies = np.ascontiguousarray(proxies, dtype=np.float32)
    psel = np.ascontiguousarray(proxies[labels])        # indexing only
    ident = np.eye(128, dtype=np.float32).astype(ml_dtypes.bfloat16)
    ppad = np.zeros((N_CORES * SH, D), dtype=np.float32)
    ppad[:C] = proxies
    in_maps = []
    for i in range(N_CORES):
        shard = ppad[i * SH:(i + 1) * SH]
        # tile-major: [c%128, t, d] so DMA chunks are per-partition contiguous
        shard_tm = np.ascontiguousarray(
            shard.reshape(NT, 128, D).transpose(1, 0, 2))
        in_maps.append({
            "pshard": shard_tm,
            "batch": batch,
            "psel": psel,
            "ident": ident,
        })
    return in_maps


def _get_nc():
    if "nc" not in _CACHE:
        _CACHE["nc"] = build_graph()
    return _CACHE["nc"]


def kernel(batch, labels, proxies):
    nc = _get_nc()
    in_maps = make_in_maps(batch, labels, proxies)
    try:
        res = run_bass_kernel_spmd(nc, in_maps, core_ids=list(range(N_CORES)))
    except Exception:
        # transient device hiccup: retry once
        res = run_bass_kernel_spmd(nc, in_maps, core_ids=list(range(N_CORES)))
    return np.float32(res.results[0]["out"][0, 0])


if __name__ == "__main__":
    rng = np.random.default_rng(0)
    batch = rng.standard_normal((B, D)).astype(np.float32)
    labels = rng.integers(0, C, B).astype(np.int64)
    proxies = (rng.standard_normal((C, D)).astype(np.float32) / 8)
    out = kernel(batch=batch, labels=labels, proxies=proxies)
    print("loss:", out)


# revision 13
# speedup vs baseline: 1.0129x; 1.0129x over previous
"""Distributed Trainium2 kernel for nn_Criterion_35012573397697.

Proxy-NCA-style loss: mean_b[ d(x_b, p_{y_b}) + logsumexp_{c != y_b}(-d(x_b, p_c)) ]
with x = 3*l2norm(batch), p = 3*l2norm(proxies), d = squared euclidean.

Strategy (8 NeuronCores, classes sharded; d(x,p) = 18 - 2*s with s = (3x^)(3p^)):
  - pshard is shipped host-side in tile-major layout [c%128, t, d] so each
    DMA chunk is per-partition contiguous (full-rate HBM streaming).
  - 7 chunks of 16/12 tiles, each chunk aligned 1:1 with an "exp group"
    (2048/1536 classes x 4 b-tiles) so the whole pipeline overlaps:
    gpsimd squares -> DVE reduce/rsqrt -> DVE bf16 copy + scale (2x modes)
    -> PE transpose (psum bank pool) -> DVE drain -> PE matmuls (512-col
    slabs into a dedicated double-buffered exp psum pool) -> ScalarE
    exp(2s-18) with fused accumulation.
  - One tiny AllGather of [128, 4] partial sums; every core computes the
    final scalar identically. Positive-class distance computed exactly in
    f32 from host-gathered proxies[labels] rows (indexing only).
"""

import math

import numpy as np
import ml_dtypes

import concourse.bass as bass
import concourse.bacc as bacc
import concourse.mybir as mybir
import concourse.tile as tile
from concourse import bass_isa
from concourse.bass_utils import run_bass_kernel_spmd

N_CORES = 8
B = 512
D = 128
C = 100000
SH = 12800           # padded shard size per core
NT = SH // 128       # 100 c-tiles of 128
BT = B // 128        # 4 b-tiles
PAD_ROWS = N_CORES * SH - C   # 2400 zero rows in total
PAD_CORR = PAD_ROWS * math.exp(-18.0)

# chunk sizes in tiles; exp groups are 12 tiles (1536 classes), last 4-tile tail
TCHUNKS = [12, 12, 12, 12, 12, 12, 12, 16]
GBOUND = [0, 12, 24, 36, 48, 60, 72, 84, 96, 100]
NG = len(GBOUND) - 1   # 9 groups per b-tile

F32 = mybir.dt.float32
BF16 = mybir.dt.bfloat16
AX = mybir.AxisListType
OP = mybir.AluOpType
AF = mybir.ActivationFunctionType

_CACHE = {}


def _rsqrt_dve(nc, pool, dst, src, n, scale=1.0, steps=2):
    """dst = scale / sqrt(src) via Quake III bit trick + Newton steps."""
    I32 = mybir.dt.int32
    v = pool.tile([128, n], F32, tag="rsq_v")
    nc.vector.tensor_scalar(v[:], src, 1e-12, None, OP.max)
    src = v[:]
    h = pool.tile([128, n], I32, tag="rsq_h")
    nc.vector.tensor_scalar(h[:], src.bitcast(I32), 1, None,
                            OP.logical_shift_right)
    y0 = pool.tile([128, n], I32, tag="rsq_y0")
    nc.vector.tensor_scalar(y0[:], h[:], -1, 0x5F3759DF, OP.mult, OP.add)
    y0f = y0[:].bitcast(F32)
    t = pool.tile([128, n], F32, tag="rsq_t")
    if steps == 1:
        nc.vector.tensor_tensor(t[:], y0f, y0f, OP.mult)    # y0^2
        nc.vector.tensor_tensor(t[:], t[:], src, OP.mult)   # v*y0^2
        nc.vector.tensor_scalar(t[:], t[:], -0.5 * scale, 1.5 * scale,
                                OP.mult, OP.add)
        nc.vector.tensor_tensor(dst, y0f, t[:], OP.mult)    # Newton 1
        return
    y1 = pool.tile([128, n], F32, tag="rsq_y1")
    nc.vector.tensor_tensor(t[:], y0f, y0f, OP.mult)        # y0^2
    nc.vector.tensor_tensor(t[:], t[:], src, OP.mult)       # v*y0^2
    nc.vector.tensor_scalar(t[:], t[:], -0.5, 1.5, OP.mult, OP.add)
    nc.vector.tensor_tensor(y1[:], y0f, t[:], OP.mult)      # Newton 1
    nc.vector.tensor_tensor(t[:], y1[:], y1[:], OP.mult)    # y1^2
    nc.vector.tensor_tensor(t[:], t[:], src, OP.mult)       # v*y1^2
    nc.vector.tensor_scalar(t[:], t[:], -0.5 * scale, 1.5 * scale,
                            OP.mult, OP.add)
    nc.vector.tensor_tensor(dst, y1[:], t[:], OP.mult)      # Newton 2


def build_graph():
    nc = bacc.Bacc("TRN2", target_bir_lowering=False, debug=False,
                   num_devices=N_CORES)
    # tile-major proxy shard: [c%128, t, d]
    p_ext = nc.dram_tensor("pshard", [128, NT, D], F32, kind="ExternalInput").ap()
    b_ext = nc.dram_tensor("batch", [B, D], F32, kind="ExternalInput").ap()
    sel_ext = nc.dram_tensor("psel", [B, D], F32, kind="ExternalInput").ap()
    id_ext = nc.dram_tensor("ident", [128, 128], BF16, kind="ExternalInput").ap()
    out_ext = nc.dram_tensor("out", [1, 1], F32, kind="ExternalOutput").ap()

    gcum = GBOUND

    with tile.TileContext(nc) as tc:
        with tc.tile_pool(name="dram", bufs=1, space="DRAM") as dram, \
             tc.tile_pool(name="big", bufs=1) as bigp, \
             tc.tile_pool(name="sb", bufs=2) as pool, \
             tc.tile_pool(name="psA", bufs=2, space="PSUM") as psA, \
             tc.tile_pool(name="psT", bufs=2, space="PSUM") as psT:

            # ---------- DMA loads: xb + dummy-collective seed first ----------
            xb = bigp.tile([128, BT, 128], F32)         # [b%128, bt, d]
            nc.sync.dma_start(xb[:], b_ext.rearrange("(t p) d -> p t d", p=128))

            dag_in = dram.tile([1, 16], F32)
            dag_out = dram.tile([N_CORES, 16], F32)
            z16 = bigp.tile([1, 16], F32)
            nc.vector.memset(z16[:], 0.0)
            dagj = bigp.tile([1, 1], F32)
            nc.sync.dma_start(dag_in[:], z16[:])
            nc.gpsimd.collective_compute(
                "AllGather", OP.bypass,
                replica_groups=[list(range(N_CORES))],
                ins=[dag_in.opt()], outs=[dag_out.opt()],
            )
            nc.sync.dma_start(dagj[:], dag_out[0:1, 0:1])

            praw = bigp.tile([128, NT, 128], F32)       # [c%128, t, d]
            nc.sync.dma_start(praw[:, 0:TCHUNKS[0], :], p_ext[:, 0:TCHUNKS[0], :])

            ident = bigp.tile([128, 128], BF16)
            nc.sync.dma_start(ident[:], id_ext[:])
            selb = bigp.tile([128, BT, 128], F32)
            nc.sync.dma_start(selb[:], sel_ext.rearrange("(t p) d -> p t d", p=128))

            clo = TCHUNKS[0]
            for ck in TCHUNKS[1:]:
                nc.sync.dma_start(praw[:, clo:clo + ck, :],
                                  p_ext[:, clo:clo + ck, :])
                clo += ck

            bias18 = bigp.tile([128, 1], F32)
            nc.vector.memset(bias18[:], -18.0)

            # preload both activation table sets off the critical path
            jnk = pool.tile([1, 2], F32, tag="smallscr")
            nc.scalar.activation(jnk[:], z16[0:1, 0:2], AF.Ln)
            nc.scalar.activation(jnk[:], z16[0:1, 0:2], AF.Exp)

            # ---------- minimal x path (xT needed by first matmul) ----------
            n2 = bigp.tile([128, 2 * BT], F32)
            sqx = pool.tile([128, BT, 128], F32, tag="sqscr")
            nc.vector.tensor_tensor(sqx[:], xb[:], xb[:], OP.mult)
            nc.vector.tensor_reduce(n2[:, 0:BT], sqx[:], axis=AX.X, op=OP.add)
            rn = bigp.tile([128, 2 * BT], F32)   # 1/sqrt(n2)
            _rsqrt_dve(nc, pool, rn[:, 0:BT], n2[:, 0:BT], BT)

            xscale3 = bigp.tile([128, BT], F32)
            nc.vector.tensor_scalar_mul(xscale3[:], rn[:, 0:BT], 3.0)
            xhat = bigp.tile([128, BT, 128], BF16)
            for t in range(BT):
                nc.vector.tensor_scalar_mul(xhat[:, t, :], xb[:, t, :],
                                            xscale3[:, t:t + 1])
            xT = bigp.tile([128, BT, 128], BF16)
            xps = psT.tile([128, 8, 128], BF16, tag="tp")
            for t in range(BT):
                nc.tensor.transpose(xps[:, t, :], xhat[:, t, :], ident[:])
            nc.vector.tensor_copy(xT[:], xps[:, 0:BT, :])

            posdot = bigp.tile([128, BT], F32)
            posd = bigp.tile([128, BT], F32)

            def posd_block():
                # deferred: only needed at the finale
                sqs = pool.tile([128, BT, 128], F32, tag="sqscr")
                nc.vector.tensor_tensor(sqs[:], selb[:], selb[:], OP.mult)
                nc.vector.tensor_reduce(n2[:, BT:2 * BT], sqs[:], axis=AX.X,
                                        op=OP.add)
                _rsqrt_dve(nc, pool, rn[:, BT:2 * BT], n2[:, BT:2 * BT], BT)
                sqd = pool.tile([128, BT, 128], F32, tag="sqscr")
                nc.vector.tensor_tensor(sqd[:], xb[:], selb[:], OP.mult)
                nc.vector.tensor_reduce(posdot[:], sqd[:], axis=AX.X,
                                        op=OP.add)
                tmp4 = pool.tile([128, BT], F32, tag="smallscr")
                nc.vector.tensor_tensor(tmp4[:], posdot[:], rn[:, 0:BT],
                                        OP.mult)
                nc.vector.tensor_tensor(tmp4[:], tmp4[:], rn[:, BT:2 * BT],
                                        OP.mult)
                nc.vector.tensor_scalar(posd[:], tmp4[:], -18.0, 18.0,
                                        OP.mult, OP.add)

            # ---------- per-chunk proxy pipeline ----------
            pn2 = bigp.tile([128, NT], F32)
            pscale3 = bigp.tile([128, NT], F32)
            pbf = bigp.tile([128, NT, 128], BF16)     # 3*normalized, bf16
            pT = bigp.tile([128, NT, 128], BF16)      # [d, t, c%128]
            partials = bigp.tile([128, BT * NG], F32)
            pTf = pT[:].rearrange("p t c -> p (t c)")

            def issue_group(g, bt):
                lo_c, hi_c = gcum[g] * 128, gcum[g + 1] * 128
                width = hi_c - lo_c
                sp = psA.tile([128, 1536], F32, tag="exp")
                for j in range(width // 512):
                    nc.tensor.matmul(
                        sp[:, j * 512:(j + 1) * 512],
                        xT[:, bt, :],
                        pTf[:, lo_c + j * 512: lo_c + (j + 1) * 512],
                        start=True, stop=True)
                ej = pool.tile([128, 1536], BF16, tag="ejunk")
                nc.scalar.activation(
                    ej[:, 0:width], sp[:, 0:width], AF.Exp,
                    bias=bias18[:, 0:1], scale=2.0,
                    accum_out=partials[:, bt * NG + g:bt * NG + g + 1])

            clo = 0
            for gi, ck in enumerate(TCHUNKS):
                lo, hi = clo, clo + ck
                clo += ck
                # squares on gpsimd (frees DVE)
                psq = pool.tile([128, 16, 128], F32, tag="psq")
                nc.gpsimd.tensor_tensor(psq[:, 0:ck, :], praw[:, lo:hi, :],
                                        praw[:, lo:hi, :], OP.mult)
                nc.vector.tensor_reduce(pn2[:, lo:hi], psq[:, 0:ck, :],
                                        axis=AX.X, op=OP.add)
                _rsqrt_dve(nc, pool, pscale3[:, lo:hi], pn2[:, lo:hi], ck,
                           scale=3.0, steps=1)
                # f32 -> bf16 copy at 2x, then bf16 scale at 2x
                pbraw = pool.tile([128, 16, 128], BF16, tag="pbraw")
                nc.vector.tensor_copy(pbraw[:, 0:ck, :], praw[:, lo:hi, :])
                nc.vector.tensor_tensor(
                    pbf[:, lo:hi, :], pbraw[:, 0:ck, :],
                    pscale3[:, lo:hi, None].to_broadcast((128, ck, 128)),
                    OP.mult)
                # transpose via PE in groups of 8 tiles, drain on DVE
                for g0 in range(lo, hi, 8):
                    w = min(8, hi - g0)
                    tp = psT.tile([128, 8, 128], BF16, tag="tp")
                    for j in range(w):
                        nc.tensor.transpose(tp[:, j, :], pbf[:, g0 + j, :],
                                            ident[:])
                    nc.vector.tensor_copy(pT[:, g0:g0 + w, :], tp[:, 0:w, :])
                # matmuls + fused exp/accum for ready groups
                glist = [gi] if gi < 7 else [7, 8]
                for g in glist:
                    for bt in range(BT):
                        issue_group(g, bt)
                if gi == 1:
                    posd_block()

            # ---------- local partial sums, tiny AllReduce ----------
            s_loc = bigp.tile([128, BT], F32)
            nc.vector.tensor_reduce(
                s_loc[:], partials[:].rearrange("p (t g) -> p t g", t=BT),
                axis=AX.X, op=OP.add)

            ar_in = dram.tile([128, BT], F32)
            ar_out = dram.tile([128, BT], F32)
            nc.sync.dma_start(ar_in[:], s_loc[:])
            nc.gpsimd.collective_compute(
                "AllReduce", OP.add,
                replica_groups=[list(range(N_CORES))],
                ins=[ar_in.opt()], outs=[ar_out.opt()],
            )
            s_tot = bigp.tile([128, BT], F32)
            nc.sync.dma_start(s_tot[:], ar_out[:])

            # ---------- finale (identical on every core) ----------
            npos = pool.tile([128, BT], F32, tag="fin")
            nc.scalar.activation(npos[:], posd[:], AF.Exp, scale=-1.0)
            s1 = pool.tile([128, BT], F32, tag="fin")
            nc.vector.tensor_scalar(s1[:], s_tot[:], -float(PAD_CORR),
                                    None, OP.add)
            nc.vector.tensor_tensor(s1[:], s1[:], npos[:], OP.subtract)
            lse = pool.tile([128, BT], F32, tag="fin")
            nc.scalar.activation(lse[:], s1[:], AF.Ln)
            perb = pool.tile([128, BT], F32, tag="fin")
            nc.vector.tensor_tensor(perb[:], posd[:], lse[:], OP.add)
            nc.vector.tensor_tensor(perb[0:1, 0:1], perb[0:1, 0:1],
                                    dagj[:], OP.add)
            ones = pool.tile([128, 1], F32, tag="fin")
            nc.vector.memset(ones[:], 1.0)
            spf = psA.tile([128, 1536], F32, tag="exp")
            nc.tensor.matmul(spf[0:1, 0:BT], ones[:], perb[:], start=True,
                             stop=True)
            csum = pool.tile([1, BT], F32, tag="fin2")
            nc.vector.tensor_copy(csum[:], spf[0:1, 0:BT])
            res = pool.tile([1, 1], F32, tag="fin3")
            nc.vector.tensor_reduce(res[:], csum[:], axis=AX.X, op=OP.add)
            nc.vector.tensor_scalar_mul(res[:], res[:], 1.0 / B)
            nc.sync.dma_start(out_ext[:], res[:])

    nc.compile()
    return nc


def make_in_maps(batch, labels, proxies):
    batch = np.ascontiguousarray(batch, dtype=np.float32)
    labels = np.asarray(labels).astype(np.int64)
    proxies = np.ascontiguousarray(proxies, dtype=np.float32)
    psel = np.ascontiguousarray(proxies[labels])        # indexing only
    ident = np.eye(128, dtype=np.float32).astype(ml_dtypes.bfloat16)
    ppad = np.zeros((N_CORES * SH, D), dtype=np.float32)
    ppad[:C] = proxies
    in_maps = []
    for i in range(N_CORES):
        shard = ppad[i * SH:(i + 1) * SH]
        # tile-major: [c%128, t, d] so DMA chunks are per-partition contiguous
        shard_tm = np.ascontiguousarray(
            shard.reshape(NT, 128, D).transpose(1, 0, 2))
        in_maps.append({
            "pshard": shard_tm,
            "batch": batch,
            "psel": psel,
            "ident": ident,
        })
    return in_maps


def _get_nc():
    if "nc" not in _CACHE:
        _CACHE["nc"] = build_graph()
    return _CACHE["nc"]


def kernel(batch, labels, proxies):
    nc = _get_nc()
    in_maps = make_in_maps(batch, labels, proxies)
    try:
        res = run_bass_kernel_spmd(nc, in_maps, core_ids=list(range(N_CORES)))
    except Exception:
        # transient device hiccup: retry once
        res = run_bass_kernel_spmd(nc, in_maps, core_ids=list(range(N_CORES)))
    return np.float32(res.results[0]["out"][0, 0])


if __name__ == "__main__":
    rng = np.random.default_rng(0)
    batch = rng.standard_normal((B, D)).astype(np.float32)
    labels = rng.integers(0, C, B).astype(np.int64)
    proxies = (rng.standard_normal((C, D)).astype(np.float32) / 8)
    out = kernel(batch=batch, labels=labels, proxies=proxies)
    print("loss:", out)


# revision 15
# speedup vs baseline: 1.1581x; 1.1433x over previous
"""Distributed Trainium2 kernel for nn_Criterion_35012573397697.

Proxy-NCA-style loss: mean_b[ d(x_b, p_{y_b}) + logsumexp_{c != y_b}(-d(x_b, p_c)) ]
with x = 3*l2norm(batch), p = 3*l2norm(proxies), d = squared euclidean.

Strategy (8 NeuronCores, classes sharded; d(x,p) = 18 - 2*s with s = (3x^)(3p^)):
  - pshard is shipped host-side in tile-major layout [c%128, t, d] so each
    DMA chunk is per-partition contiguous (full-rate HBM streaming).
  - 7 chunks of 16/12 tiles, each chunk aligned 1:1 with an "exp group"
    (2048/1536 classes x 4 b-tiles) so the whole pipeline overlaps:
    gpsimd squares -> DVE reduce/rsqrt -> DVE bf16 copy + scale (2x modes)
    -> PE transpose (psum bank pool) -> DVE drain -> PE matmuls (512-col
    slabs into a dedicated double-buffered exp psum pool) -> ScalarE
    exp(2s-18) with fused accumulation.
  - One tiny AllGather of [128, 4] partial sums; every core computes the
    final scalar identically. Positive-class distance computed exactly in
    f32 from host-gathered proxies[labels] rows (indexing only).
"""

import math

import numpy as np
import ml_dtypes

import concourse.bass as bass
import concourse.bacc as bacc
import concourse.mybir as mybir
import concourse.tile as tile
from concourse import bass_isa
from concourse.bass_utils import run_bass_kernel_spmd

N_CORES = 8
B = 512
D = 128
C = 100000
SH = 12800           # padded shard size per core
NT = SH // 128       # 100 c-tiles of 128
BT = B // 128        # 4 b-tiles
PAD_ROWS = N_CORES * SH - C   # 2400 zero rows in total
PAD_CORR = PAD_ROWS * math.exp(-18.0)

# chunk sizes in tiles; exp groups are 12 tiles (1536 classes), last 4-tile tail
TCHUNKS = [12, 12, 12, 12, 12, 12, 12, 16]
GBOUND = [0, 12, 24, 36, 48, 60, 72, 84, 96, 100]
NG = len(GBOUND) - 1   # 9 groups per b-tile

F32 = mybir.dt.float32
BF16 = mybir.dt.bfloat16
AX = mybir.AxisListType
OP = mybir.AluOpType
AF = mybir.ActivationFunctionType

_CACHE = {}


def _rsqrt_dve(nc, pool, dst, src, n, scale=1.0, steps=2):
    """dst = scale / sqrt(src) via Quake III bit trick + Newton steps."""
    I32 = mybir.dt.int32
    v = pool.tile([128, n], F32, tag="rsq_v")
    nc.vector.tensor_scalar(v[:], src, 1e-12, None, OP.max)
    src = v[:]
    h = pool.tile([128, n], I32, tag="rsq_h")
    nc.vector.tensor_scalar(h[:], src.bitcast(I32), 1, None,
                            OP.logical_shift_right)
    y0 = pool.tile([128, n], I32, tag="rsq_y0")
    nc.vector.tensor_scalar(y0[:], h[:], -1, 0x5F3759DF, OP.mult, OP.add)
    y0f = y0[:].bitcast(F32)
    t = pool.tile([128, n], F32, tag="rsq_t")
    if steps == 1:
        nc.vector.tensor_tensor(t[:], y0f, y0f, OP.mult)    # y0^2
        nc.vector.tensor_tensor(t[:], t[:], src, OP.mult)   # v*y0^2
        nc.vector.tensor_scalar(t[:], t[:], -0.5 * scale, 1.5 * scale,
                                OP.mult, OP.add)
        nc.vector.tensor_tensor(dst, y0f, t[:], OP.mult)    # Newton 1
        return
    y1 = pool.tile([128, n], F32, tag="rsq_y1")
    nc.vector.tensor_tensor(t[:], y0f, y0f, OP.mult)        # y0^2
    nc.vector.tensor_tensor(t[:], t[:], src, OP.mult)       # v*y0^2
    nc.vector.tensor_scalar(t[:], t[:], -0.5, 1.5, OP.mult, OP.add)
    nc.vector.tensor_tensor(y1[:], y0f, t[:], OP.mult)      # Newton 1
    nc.vector.tensor_tensor(t[:], y1[:], y1[:], OP.mult)    # y1^2
    nc.vector.tensor_tensor(t[:], t[:], src, OP.mult)       # v*y1^2
    nc.vector.tensor_scalar(t[:], t[:], -0.5 * scale, 1.5 * scale,
                            OP.mult, OP.add)
    nc.vector.tensor_tensor(dst, y1[:], t[:], OP.mult)      # Newton 2


def build_graph():
    nc = bacc.Bacc("TRN2", target_bir_lowering=False, debug=False,
                   num_devices=N_CORES)
    # tile-major proxy shard: [c%128, t, d]
    p_ext = nc.dram_tensor("pshard", [128, NT, D], F32, kind="ExternalInput").ap()
    b_ext = nc.dram_tensor("batch", [B, D], F32, kind="ExternalInput").ap()
    sel_ext = nc.dram_tensor("psel", [B, D], F32, kind="ExternalInput").ap()
    id_ext = nc.dram_tensor("ident", [128, 128], BF16, kind="ExternalInput").ap()
    out_ext = nc.dram_tensor("out", [1, 1], F32, kind="ExternalOutput").ap()

    gcum = GBOUND

    with tile.TileContext(nc) as tc:
        with tc.tile_pool(name="dram", bufs=1, space="DRAM") as dram, \
             tc.tile_pool(name="big", bufs=1) as bigp, \
             tc.tile_pool(name="sb", bufs=2) as pool, \
             tc.tile_pool(name="psA", bufs=2, space="PSUM") as psA, \
             tc.tile_pool(name="psT", bufs=2, space="PSUM") as psT:

            # ---------- DMA loads: xb + dummy-collective seed first ----------
            xb = bigp.tile([128, BT, 128], F32)         # [b%128, bt, d]
            nc.sync.dma_start(xb[:], b_ext.rearrange("(t p) d -> p t d", p=128))

            dag_in = dram.tile([1, 16], F32)
            dag_out = dram.tile([N_CORES, 16], F32)
            z16 = bigp.tile([1, 16], F32)
            nc.vector.memset(z16[:], 0.0)
            dagj = bigp.tile([1, 1], F32)
            nc.sync.dma_start(dag_in[:], z16[:])
            nc.gpsimd.collective_compute(
                "AllGather", OP.bypass,
                replica_groups=[list(range(N_CORES))],
                ins=[dag_in.opt()], outs=[dag_out.opt()],
            )
            # NOTE: dag_out is read back only in the finale — a readback DMA
            # here would sit pending in the HWDGE FIFO and stall every later
            # proxy-chunk DMA behind the collective.

            praw = bigp.tile([128, NT, 128], F32)       # [c%128, t, d]
            nc.sync.dma_start(praw[:, 0:TCHUNKS[0], :], p_ext[:, 0:TCHUNKS[0], :])

            ident = bigp.tile([128, 128], BF16)
            nc.sync.dma_start(ident[:], id_ext[:])
            selb = bigp.tile([128, BT, 128], F32)
            nc.sync.dma_start(selb[:], sel_ext.rearrange("(t p) d -> p t d", p=128))

            clo = TCHUNKS[0]
            for ck in TCHUNKS[1:]:
                nc.sync.dma_start(praw[:, clo:clo + ck, :],
                                  p_ext[:, clo:clo + ck, :])
                clo += ck

            bias18 = bigp.tile([128, 1], F32)
            nc.vector.memset(bias18[:], -18.0)

            # preload both activation table sets off the critical path
            jnk = pool.tile([1, 2], F32, tag="smallscr")
            nc.scalar.activation(jnk[:], z16[0:1, 0:2], AF.Ln)
            nc.scalar.activation(jnk[:], z16[0:1, 0:2], AF.Exp)

            # ---------- minimal x path (xT needed by first matmul) ----------
            n2 = bigp.tile([128, 2 * BT], F32)
            sqx = pool.tile([128, BT, 128], F32, tag="sqscr")
            nc.vector.tensor_tensor(sqx[:], xb[:], xb[:], OP.mult)
            nc.vector.tensor_reduce(n2[:, 0:BT], sqx[:], axis=AX.X, op=OP.add)
            rn = bigp.tile([128, 2 * BT], F32)   # 1/sqrt(n2)
            _rsqrt_dve(nc, pool, rn[:, 0:BT], n2[:, 0:BT], BT)

            xscale3 = bigp.tile([128, BT], F32)
            nc.vector.tensor_scalar_mul(xscale3[:], rn[:, 0:BT], 3.0)
            xhat = bigp.tile([128, BT, 128], BF16)
            for t in range(BT):
                nc.vector.tensor_scalar_mul(xhat[:, t, :], xb[:, t, :],
                                            xscale3[:, t:t + 1])
            xT = bigp.tile([128, BT, 128], BF16)
            xps = psT.tile([128, 8, 128], BF16, tag="tp")
            for t in range(BT):
                nc.tensor.transpose(xps[:, t, :], xhat[:, t, :], ident[:])
            nc.vector.tensor_copy(xT[:], xps[:, 0:BT, :])

            posdot = bigp.tile([128, BT], F32)
            posd = bigp.tile([128, BT], F32)

            def posd_block():
                # deferred: only needed at the finale
                sqs = pool.tile([128, BT, 128], F32, tag="sqscr")
                nc.vector.tensor_tensor(sqs[:], selb[:], selb[:], OP.mult)
                nc.vector.tensor_reduce(n2[:, BT:2 * BT], sqs[:], axis=AX.X,
                                        op=OP.add)
                _rsqrt_dve(nc, pool, rn[:, BT:2 * BT], n2[:, BT:2 * BT], BT)
                sqd = pool.tile([128, BT, 128], F32, tag="sqscr")
                nc.vector.tensor_tensor(sqd[:], xb[:], selb[:], OP.mult)
                nc.vector.tensor_reduce(posdot[:], sqd[:], axis=AX.X,
                                        op=OP.add)
                tmp4 = pool.tile([128, BT], F32, tag="smallscr")
                nc.vector.tensor_tensor(tmp4[:], posdot[:], rn[:, 0:BT],
                                        OP.mult)
                nc.vector.tensor_tensor(tmp4[:], tmp4[:], rn[:, BT:2 * BT],
                                        OP.mult)
                nc.vector.tensor_scalar(posd[:], tmp4[:], -18.0, 18.0,
                                        OP.mult, OP.add)

            # ---------- per-chunk proxy pipeline ----------
            pn2 = bigp.tile([128, NT], F32)
            pscale3 = bigp.tile([128, NT], F32)
            pbf = bigp.tile([128, NT, 128], BF16)     # 3*normalized, bf16
            pT = bigp.tile([128, NT, 128], BF16)      # [d, t, c%128]
            partials = bigp.tile([128, BT * NG], F32)
            pTf = pT[:].rearrange("p t c -> p (t c)")

            def issue_group(g, bt):
                lo_c, hi_c = gcum[g] * 128, gcum[g + 1] * 128
                width = hi_c - lo_c
                sp = psA.tile([128, 1536], F32, tag="exp")
                for j in range(width // 512):
                    nc.tensor.matmul(
                        sp[:, j * 512:(j + 1) * 512],
                        xT[:, bt, :],
                        pTf[:, lo_c + j * 512: lo_c + (j + 1) * 512],
                        start=True, stop=True)
                ej = pool.tile([128, 1536], BF16, tag="ejunk")
                nc.scalar.activation(
                    ej[:, 0:width], sp[:, 0:width], AF.Exp,
                    bias=bias18[:, 0:1], scale=2.0,
                    accum_out=partials[:, bt * NG + g:bt * NG + g + 1])

            clo = 0
            for gi, ck in enumerate(TCHUNKS):
                lo, hi = clo, clo + ck
                clo += ck
                # squares on gpsimd (frees DVE)
                psq = pool.tile([128, 16, 128], F32, tag="psq")
                nc.gpsimd.tensor_tensor(psq[:, 0:ck, :], praw[:, lo:hi, :],
                                        praw[:, lo:hi, :], OP.mult)
                nc.vector.tensor_reduce(pn2[:, lo:hi], psq[:, 0:ck, :],
                                        axis=AX.X, op=OP.add)
                _rsqrt_dve(nc, pool, pscale3[:, lo:hi], pn2[:, lo:hi], ck,
                           scale=3.0, steps=1)
                # f32 -> bf16 copy at 2x, then bf16 scale at 2x
                pbraw = pool.tile([128, 16, 128], BF16, tag="pbraw")
                nc.vector.tensor_copy(pbraw[:, 0:ck, :], praw[:, lo:hi, :])
                nc.vector.tensor_tensor(
                    pbf[:, lo:hi, :], pbraw[:, 0:ck, :],
                    pscale3[:, lo:hi, None].to_broadcast((128, ck, 128)),
                    OP.mult)
                # transpose via PE in groups of 8 tiles, drain on DVE
                for g0 in range(lo, hi, 8):
                    w = min(8, hi - g0)
                    tp = psT.tile([128, 8, 128], BF16, tag="tp")
                    for j in range(w):
                        nc.tensor.transpose(tp[:, j, :], pbf[:, g0 + j, :],
                                            ident[:])
                    nc.vector.tensor_copy(pT[:, g0:g0 + w, :], tp[:, 0:w, :])
                # matmuls + fused exp/accum for ready groups
                glist = [gi] if gi < 7 else [7, 8]
                for g in glist:
                    for bt in range(BT):
                        issue_group(g, bt)
                if gi == 1:
                    posd_block()

            # ---------- local partial sums, tiny AllReduce ----------
            s_loc = bigp.tile([128, BT], F32)
            nc.vector.tensor_reduce(
                s_loc[:], partials[:].rearrange("p (t g) -> p t g", t=BT),
                axis=AX.X, op=OP.add)

            nc.sync.dma_start(dagj[:], dag_out[0:1, 0:1])
            ar_in = dram.tile([128, BT], F32)
            ar_out = dram.tile([128, BT], F32)
            nc.sync.dma_start(ar_in[:], s_loc[:])
            nc.gpsimd.collective_compute(
                "AllReduce", OP.add,
                replica_groups=[list(range(N_CORES))],
                ins=[ar_in.opt()], outs=[ar_out.opt()],
            )
            s_tot = bigp.tile([128, BT], F32)
            nc.sync.dma_start(s_tot[:], ar_out[:])

            # ---------- finale (identical on every core) ----------
            npos = pool.tile([128, BT], F32, tag="fin")
            nc.scalar.activation(npos[:], posd[:], AF.Exp, scale=-1.0)
            s1 = pool.tile([128, BT], F32, tag="fin")
            nc.vector.tensor_scalar(s1[:], s_tot[:], -float(PAD_CORR),
                                    None, OP.add)
            nc.vector.tensor_tensor(s1[:], s1[:], npos[:], OP.subtract)
            lse = pool.tile([128, BT], F32, tag="fin")
            nc.scalar.activation(lse[:], s1[:], AF.Ln)
            perb = pool.tile([128, BT], F32, tag="fin")
            nc.vector.tensor_tensor(perb[:], posd[:], lse[:], OP.add)
            nc.vector.tensor_tensor(perb[0:1, 0:1], perb[0:1, 0:1],
                                    dagj[:], OP.add)
            ones = pool.tile([128, 1], F32, tag="fin")
            nc.vector.memset(ones[:], 1.0)
            spf = psA.tile([128, 1536], F32, tag="exp")
            nc.tensor.matmul(spf[0:1, 0:BT], ones[:], perb[:], start=True,
                             stop=True)
            csum = pool.tile([1, BT], F32, tag="fin2")
            nc.vector.tensor_copy(csum[:], spf[0:1, 0:BT])
            res = pool.tile([1, 1], F32, tag="fin3")
            nc.vector.tensor_reduce(res[:], csum[:], axis=AX.X, op=OP.add)
            nc.vector.tensor_scalar_mul(res[:], res[:], 1.0 / B)
            nc.sync.dma_start(out_ext[:], res[:])

    nc.compile()
    return nc


def make_in_maps(batch, labels, proxies):
    batch = np.ascontiguousarray(batch, dtype=np.float32)
    labels = np.asarray(labels).astype(np.int64)
    proxies = np.ascontiguousarray(proxies, dtype=np.float32)
    psel = np.ascontiguousarray(proxies[labels])        # indexing only
    ident = np.eye(128, dtype=np.float32).astype(ml_dtypes.bfloat16)
    ppad = np.zeros((N_CORES * SH, D), dtype=np.float32)
    ppad[:C] = proxies
    in_maps = []
    for i in range(N_CORES):
        shard = ppad[i * SH:(i + 1) * SH]
        # tile-major: [c%128, t, d] so DMA chunks are per-partition contiguous
        shard_tm = np.ascontiguousarray(
            shard.reshape(NT, 128, D).transpose(1, 0, 2))
        in_maps.append({
            "pshard": shard_tm,
            "batch": batch,
            "psel": psel,
            "ident": ident,
        })
    return in_maps


def _get_nc():
    if "nc" not in _CACHE:
        _CACHE["nc"] = build_graph()
    return _CACHE["nc"]


def kernel(batch, labels, proxies):
    nc = _get_nc()
    in_maps = make_in_maps(batch, labels, proxies)
    try:
        res = run_bass_kernel_spmd(nc, in_maps, core_ids=list(range(N_CORES)))
    except Exception:
        # transient device hiccup: retry once
        res = run_bass_kernel_spmd(nc, in_maps, core_ids=list(range(N_CORES)))
    return np.float32(res.results[0]["out"][0, 0])


if __name__ == "__main__":
    rng = np.random.default_rng(0)
    batch = rng.standard_normal((B, D)).astype(np.float32)
    labels = rng.integers(0, C, B).astype(np.int64)
    proxies = (rng.standard_normal((C, D)).astype(np.float32) / 8)
    out = kernel(batch=batch, labels=labels, proxies=proxies)
    print("loss:", out)
